# revision 21
# baseline (speedup 1.0000x reference)
"""Trainium2 Bass kernel for nn_CACBlock (multi-dilation depthwise conv +
top-k autocorrelation mixing + projection + residual LayerNorm).

Data-parallel over batch: 8 cores x 4 batches each. Full inputs in,
full outputs out; all sharding/gather inside kernel().

v2 design: everything heavy runs on PE.
 - conv branch = 11 diag matmuls per (q, chunk) accumulating in PSUM
   (diag tiles precomputed on host). Zero start-margin handles negative
   taps; clipped matmuls at the top edge handle positive taps.
 - xm row-means = ones-vector matmuls straight off the transposed bf16
   tile (kills the f32 x reload).
 - residual x add folded into the projection PSUM via identity-matmul
   transpose (kills the second f32 x reload).
 - output written bf16, cast to f32 on host.
"""

import numpy as np
import ml_dtypes

import concourse.bass as bass
import concourse.bacc as bacc
import concourse.tile as tile
import concourse.mybir as mybir
from concourse.bass_utils import run_bass_kernel_spmd

F32 = mybir.dt.float32
BF16 = mybir.dt.bfloat16
FP8 = mybir.dt.float8e4
I32 = mybir.dt.int32
U32 = mybir.dt.uint32

B, L, D = 32, 2048, 512
N_CORES = 8
B_LOC = B // N_CORES
D2 = 2 * D
TOPK = 5
MAXC = 32
LN_EPS = 1e-5
P = 128
N_DBLK = D // P          # 4 d-blocks per branch
N_KBLK = D2 // P         # 8 contraction blocks
N_LT = L // P            # 16 l-tiles per batch
MAX_N = 512              # PE moving free dim
SM = 16                  # zero start margin (covers most-negative tap)
XT2W = SM + L + MAX_N    # margin + x + wrap margin for rotations
N_CH = L // MAX_N        # 4 chunks per row


def _cand_lags() -> np.ndarray:
    max_lag = min(L - 1, 168)
    num = min(max_lag, MAXC)
    return np.linspace(1, max_lag, num).astype(np.int64)


def _tap_table(conv_w0, conv_w1, conv_w2):
    """Combined depthwise kernel: offset -> per-channel weight [D]."""
    taps = {}
    for w, dil in ((conv_w0, 1), (conv_w1, 2), (conv_w2, 4)):
        k = w.shape[-1]
        for j in range(k):
            off = (j - k // 2) * dil
            taps[off] = taps.get(off, 0.0) + w[:, 0, j].astype(np.float64)
    offs = sorted(taps)
    tab = np.stack([taps[o] for o in offs], axis=1)  # [D, n_taps]
    return offs, tab.astype(np.float32)


def _build(n_taps: int, need_pb: bool, need_gb: bool, repeat: int = 1,
           skip: frozenset = frozenset()):
    """Build + compile the per-core program (identical across cores).

    repeat>1 emits the whole batch pipeline `repeat` times (same inputs,
    same outputs) — used only by timing tools to measure steady-state
    per-iteration time through the high-overhead axon tunnel.
    skip: timing-only component ablations (output is wrong with any set):
    {"conv_mm","auto_mm","proj_mm","in_dma","out_dma","scores"}.
    """
    nc = bacc.Bacc(
        "TRN2",
        target_bir_lowering=False,
        debug=False,
        num_devices=N_CORES,
    )
    xbf_d = nc.dram_tensor("xbf", [B_LOC, D, L], BF16, kind="ExternalInput")
    pwt_d = nc.dram_tensor("pwt", [D, D], FP8, kind="ExternalInput")
    pwb_d = nc.dram_tensor("pwb", [D, D], BF16, kind="ExternalInput")
    cdg_d = nc.dram_tensor("cdg", [n_taps * N_DBLK * P, P], BF16,
                           kind="ExternalInput")
    n_pairs = (n_taps - 1) // 2
    xf8_d = nc.dram_tensor("xf8", [B_LOC, D, L], FP8, kind="ExternalInput")
    cd8_d = nc.dram_tensor("cd8", [n_pairs * N_DBLK * P, 2 * P], FP8,
                           kind="ExternalInput")
    tst_d = nc.dram_tensor("tstart", [1, MAXC], I32, kind="ExternalInput")
    icnt_d = nc.dram_tensor("invcnt", [MAXC, 1], F32, kind="ExternalInput")
    idt_d = nc.dram_tensor("idt", [P, P], BF16, kind="ExternalInput")
    if need_pb:
        pb_d = nc.dram_tensor("pb", [1, D], F32, kind="ExternalInput")
    if need_gb:
        g_d = nc.dram_tensor("lng", [1, D], F32, kind="ExternalInput")
        bb_d = nc.dram_tensor("lnb", [1, D], F32, kind="ExternalInput")
    out_d = nc.dram_tensor("out", [B_LOC, L, D], BF16, kind="ExternalOutput")

    lags = _cand_lags()
    max_lag = int(lags[-1])
    Add = mybir.AluOpType.add
    Sub = mybir.AluOpType.subtract
    Mult = mybir.AluOpType.mult
    Copy = mybir.ActivationFunctionType.Copy
    Square = mybir.ActivationFunctionType.Square
    Sqrt = mybir.ActivationFunctionType.Sqrt

    # conv tap order per chunk: offset 0 first (full range, start=True),
    # then positive offsets (clipped at top edge), negatives last (the
    # final one is always full-range and carries stop=True).
    order = (
        [j for j in range(n_taps) if tap_offsets[j] == 0]
        + [j for j in range(n_taps) if tap_offsets[j] > 0]
        + [j for j in range(n_taps) if tap_offsets[j] < 0]
    )
    # adjacent pairing of non-center taps (offsets ascending) for fp8
    # DoubleRow: 2 taps per PE pass
    noncenter = [j for j in range(n_taps) if tap_offsets[j] != 0]
    noncenter.sort(key=lambda j: tap_offsets[j])
    pairings = [(noncenter[2 * i], noncenter[2 * i + 1])
                for i in range(len(noncenter) // 2)]

    with tile.TileContext(nc) as tc:
        with (
            tc.tile_pool(name="const", bufs=1) as cpool,
            tc.tile_pool(name="xt2", bufs=8) as xtpool,
            tc.tile_pool(name="acc", bufs=8) as accpool,
            tc.tile_pool(name="aacc", bufs=5) as aaccpool,
            tc.tile_pool(name="sc", bufs=1) as scpool,
            tc.tile_pool(name="xmrp", bufs=2) as xmrpool,
            tc.tile_pool(name="tiny", bufs=4) as tpool,
            tc.tile_pool(name="junk", bufs=3) as jpool,
            tc.tile_pool(name="outp", bufs=4) as opool,
            tc.tile_pool(name="hp", bufs=18) as hpool,
            tc.tile_pool(name="dg", bufs=2) as dgpool,
            tc.tile_pool(name="psum", bufs=2, space=bass.MemorySpace.PSUM) as pspool,
            tc.tile_pool(name="cpsum", bufs=2, space=bass.MemorySpace.PSUM) as cpspool,
            tc.tile_pool(name="apsum", bufs=2, space=bass.MemorySpace.PSUM) as apspool,
            tc.tile_pool(name="xmpsum", bufs=2, space=bass.MemorySpace.PSUM) as xmpspool,
        ):
            # ---- constants (loaded once) ----
            pwt_sb = cpool.tile([P, N_DBLK, D], FP8)  # conv half [p, k, o]
            nc.sync.dma_start(
                pwt_sb[:], pwt_d[:, :].rearrange("(k p) o -> p k o", p=P)
            )
            pwb_sb = cpool.tile([P, N_DBLK, D], BF16)  # auto half [p, k, o]
            nc.sync.dma_start(
                pwb_sb[:], pwb_d[:, :].rearrange("(k p) o -> p k o", p=P)
            )
            cdg_sb = cpool.tile([P, n_taps * N_DBLK, P], BF16)  # [p, jq, m]
            nc.sync.dma_start(
                cdg_sb[:], cdg_d[:, :].rearrange("(t p) m -> p t m", p=P)
            )
            cd8_sb = cpool.tile([P, n_pairs * N_DBLK, 2, P], FP8)
            nc.sync.dma_start(
                cd8_sb[:],
                cd8_d[:, :].rearrange("(t p) (u m) -> p t u m", p=P, u=2),
            )
            tst_sb = cpool.tile([1, MAXC], I32)
            nc.sync.dma_start(tst_sb[:], tst_d[:, :])
            eps_sb = cpool.tile([P, 1], F32)
            nc.vector.memset(eps_sb[:], LN_EPS)
            ones_sb = cpool.tile([P, 1], BF16)
            nc.vector.memset(ones_sb[:], 1.0)
            idt_sb = cpool.tile([P, P], BF16)
            nc.sync.dma_start(idt_sb[:], idt_d[:, :])
            icnt_sb = cpool.tile([MAXC, 1], F32)
            nc.sync.dma_start(icnt_sb[:], icnt_d[:, :])
            if need_pb:
                pb_r = cpool.tile([1, D], F32)
                nc.sync.dma_start(pb_r[:], pb_d[:, :])
                pb_sb = cpool.tile([P, D], F32)
                nc.gpsimd.partition_broadcast(pb_sb[:], pb_r[0:1, :])
            if need_gb:
                g_r = cpool.tile([1, D], F32)
                nc.sync.dma_start(g_r[:], g_d[:, :])
                g_sb = cpool.tile([P, D], F32)
                nc.gpsimd.partition_broadcast(g_sb[:], g_r[0:1, :])
                bb_r = cpool.tile([1, D], F32)
                nc.sync.dma_start(bb_r[:], bb_d[:, :])
                bb_sb = cpool.tile([P, D], F32)
                nc.gpsimd.partition_broadcast(bb_sb[:], bb_r[0:1, :])

            def emit_A(b):
                st = {}

                # ---- transposed bf16 copy: xT2[q] = [128 d, SM+L+512 l] ----
                xt2 = []
                for q in range(N_DBLK):
                    xq = xtpool.tile([P, XT2W], BF16, tag="xt2")
                    nc.vector.memset(xq[:, 0:SM], 0.0)
                    if "in_dma" not in skip:
                        eng = (nc.sync, nc.scalar)[q % 2]
                        eng.dma_start(
                            xq[:, SM:SM + L],
                            xbf_d[b, q * P:(q + 1) * P, :],
                        )
                        eng.dma_start(
                            xq[:, SM + L:XT2W], xq[:, SM:SM + MAX_N]
                        )
                    xt2.append(xq)
                st["xt2"] = xt2

                xf8 = []
                for q in range(N_DBLK):
                    x8 = xtpool.tile([P, SM + L + SM], FP8, tag="xf8")
                    nc.vector.memset(x8[:, 0:SM], 0.0)
                    nc.vector.memset(x8[:, SM + L:SM + L + SM], 0.0)
                    if "in_dma" not in skip:
                        eng = (nc.sync, nc.scalar)[q % 2]
                        eng.dma_start(
                            x8[:, SM:SM + L], xf8_d[b, q * P:(q + 1) * P, :]
                        )
                    xf8.append(x8)

                # ---- xm row [1, L] via ones-matmul (PE), f32 psum ----
                xmr = xmrpool.tile([1, L], BF16, tag="xmr")
                for c in range(N_CH):
                    xmp = xmpspool.tile([1, MAX_N], F32, tag="xmp")
                    if "xm" not in skip:
                        for q in range(N_DBLK):
                            nc.tensor.matmul(
                                xmp[:],
                                ones_sb[:],
                                xt2[q][:, SM + c * MAX_N:SM + (c + 1) * MAX_N],
                                start=(q == 0),
                                stop=(q == N_DBLK - 1),
                            )
                    else:
                        nc.tensor.matmul(
                            xmp[0:1, 0:8], ones_sb[:], xt2[0][:, 0:8],
                            start=True, stop=True,
                        )
                    nc.scalar.activation(
                        xmr[0:1, c * MAX_N:(c + 1) * MAX_N], xmp[:], Copy
                    )

                # ---- autocorrelation scores (f32, exact) ----
                srow = tpool.tile([1, MAXC], F32, tag="srow")
                if "scores" not in skip:
                    shift = scpool.tile([MAXC, L], BF16, tag="shift")
                    nc.gpsimd.memset(shift[:, L - max_lag:L], 0.0)
                    qs = (nc.sync, nc.scalar, nc.gpsimd)
                    for i, lag in enumerate(lags):
                        lag = int(lag)
                        qs[i % 3].dma_start(
                            shift[i:i + 1, 0:L - lag], xmr[0:1, lag:L]
                        )
                    bcast = scpool.tile([MAXC, L], BF16, tag="bcast")
                    nc.gpsimd.partition_broadcast(bcast[:], xmr[0:1, :])
                    sraw = tpool.tile([MAXC, 1], F32, tag="sraw")
                    nc.vector.scalar_tensor_tensor(
                        shift[:], shift[:], 1.0, bcast[:], Mult, Mult,
                        accum_out=sraw[:],
                    )
                    scores = tpool.tile([MAXC, 1], F32, tag="scores")
                    nc.vector.tensor_scalar(
                        scores[:], sraw[:], icnt_sb[:], None, Mult
                    )
                    nc.sync.dma_start(srow[0:1, :], scores[:, :])
                else:
                    nc.vector.memset(srow[:], 0.5)

                # ---- top-5 ----
                vmax = tpool.tile([1, 8], F32, tag="vmax")
                idx8 = tpool.tile([1, 8], U32, tag="idx8")
                nc.vector.max_with_indices(vmax[:], idx8[:], srow[:])
                dsum = tpool.tile([1, 1], F32, tag="dsum")
                nc.vector.tensor_reduce(
                    dsum[:], vmax[0:1, 0:TOPK], mybir.AxisListType.X, Add
                )
                nc.vector.tensor_scalar(dsum[:], dsum[:], 1e-6, None, Add)
                dinv = tpool.tile([1, 1], F32, tag="dinv")
                nc.vector.reciprocal(dinv[:], dsum[:])
                w5 = tpool.tile([1, TOPK], F32, tag="w5")
                nc.vector.tensor_scalar(
                    w5[:], vmax[0:1, 0:TOPK], dinv[:], None, Mult
                )
                wbc = tpool.tile([P, TOPK], F32, tag="wbc")
                nc.gpsimd.partition_broadcast(wbc[:], w5[0:1, :])
                # diag(w_i) lhsT tiles for the PE auto path
                diags = []
                for i in range(TOPK):
                    dg = dgpool.tile([P, P], BF16, tag=f"diag{i}")
                    nc.vector.tensor_scalar(
                        dg[:], idt_sb[:], wbc[:, i:i + 1], None, Mult
                    )
                    diags.append(dg)
                st["diags"] = diags
                st["idx8"] = idx8

                # ---- conv branch on PE: diag matmuls into PSUM ----
                conv = []
                for qp in range(N_DBLK // 2):
                    cqp = accpool.tile([P, 2, L], FP8, tag="conv")
                    conv.append(cqp)
                for q in range(N_DBLK):
                    cq = conv[q // 2]
                    qu = q % 2
                    for c in range(N_CH):
                        cp = cpspool.tile([P, MAX_N], F32, tag="cps")
                        o0 = c * MAX_N
                        if "conv_mm" in skip:
                            nc.tensor.matmul(
                                cp[:, 0:8], cdg_sb[:, 0, :],
                                xt2[q][:, 0:8], start=True, stop=True,
                            )
                        elif "fp8" not in skip:
                            # center tap bf16 (start), then 5 fp8 DoubleRow
                            # paired-tap matmuls (2 taps per pass)
                            jc = order[0]
                            nc.tensor.matmul(
                                cp[:],
                                cdg_sb[:, jc * N_DBLK + q, :],
                                xt2[q][:, SM + o0:SM + o0 + MAX_N],
                                start=True, stop=False,
                            )
                            for pi, (j1, j2) in enumerate(pairings):
                                o1 = tap_offsets[j1]
                                delta = tap_offsets[j2] - o1
                                rhs = xf8[q][:, SM + o0 + o1:
                                             SM + o0 + o1 + MAX_N]
                                rhs = rhs.unsqueeze(1)
                                rhs.ap[1] = [delta, 2]
                                nc.tensor.matmul(
                                    cp[:],
                                    cd8_sb[:, pi * N_DBLK + q, :, :],
                                    rhs,
                                    start=False,
                                    stop=(pi == n_pairs - 1),
                                    perf_mode=mybir.MatmulPerfMode.DoubleRow,
                                )
                        else:
                            for t_i, j in enumerate(order):
                                off = tap_offsets[j]
                                width = MAX_N
                                lo = o0
                                if c == N_CH - 1 and off > 0:
                                    width = MAX_N - off
                                nc.tensor.matmul(
                                    cp[:, lo - o0:lo - o0 + width],
                                    cdg_sb[:, j * N_DBLK + q, :],
                                    xt2[q][:, SM + lo + off:
                                           SM + lo + off + width],
                                    start=(t_i == 0),
                                    stop=(t_i == n_taps - 1),
                                )
                        nc.vector.tensor_copy(
                            cq[:, qu, o0:o0 + MAX_N], cp[:]
                        )
                st["conv"] = conv
                return st

            def emit_A2(b, st):
                # auto branch on PE: psum += diag(w_i) @ x[:, rot_i] per chunk
                xt2, diags, idx8 = st["xt2"], st["diags"], st["idx8"]
                rep = st["rep"]
                starts = []
                for i in range(TOPK):
                    r1 = nc.alloc_register(mybir.EngineType.PE, f"ix{rep}_{b}_{i}")
                    nc.tensor.reg_load(r1, idx8[0:1, i:i + 1])
                    s1 = nc.snap(r1, donate=True, min_val=0, max_val=MAXC - 1)
                    r2 = nc.alloc_register(mybir.EngineType.PE, f"st{rep}_{b}_{i}")
                    nc.tensor.reg_load(r2, tst_sb[0:1, bass.ds(s1, 1)])
                    row = []
                    for c in range(N_CH):
                        r3 = nc.alloc_register(
                            mybir.EngineType.PE, f"sc{rep}_{b}_{i}_{c}"
                        )
                        nc.tensor.reg_alu(r3, r2, c * MAX_N - L, Add)
                        if c == 0:
                            nc.tensor.reg_alu(r3, r3, L, Add)
                            lo = SM + L - max_lag
                        else:
                            lo = SM + c * MAX_N - max_lag
                        s3 = nc.snap(
                            r3, donate=True, min_val=lo,
                            max_val=lo + max_lag - 1,
                        )
                        row.append(s3)
                    starts.append(row)
                auto = []
                for q in range(N_DBLK):
                    aq = aaccpool.tile([P, L], BF16, tag="auto")
                    auto.append(aq)
                for q in range(N_DBLK):
                    aq = auto[q]
                    for c in range(N_CH):
                        ap = apspool.tile([P, MAX_N], F32, tag="aps")
                        if "auto_mm" in skip:
                            nc.tensor.matmul(
                                ap[:, 0:8], diags[0][:],
                                xt2[q][:, 0:8], start=True, stop=True,
                            )
                        for i in range(TOPK):
                            if "auto_mm" in skip:
                                break
                            nc.tensor.matmul(
                                ap[:],
                                diags[i][:],
                                xt2[q][:, bass.ds(starts[i][c], MAX_N)],
                                start=(i == 0),
                                stop=(i == TOPK - 1),
                            )
                        nc.scalar.activation(
                            aq[:, c * MAX_N:(c + 1) * MAX_N], ap[:], Copy
                        )
                st["auto"] = auto
                return st

            def emit_B(b, st):
                conv, auto, xt2 = st["conv"], st["auto"], st["xt2"]
                hs = tpool.tile([P, N_LT], F32, tag="hs")
                hs2 = tpool.tile([P, N_LT], F32, tag="hs2")
                hts = []
                for t in range(N_LT):
                    ps = pspool.tile([P, D], F32, tag="ps")
                    if "proj_mm" in skip:
                        nc.tensor.matmul(
                            ps[:, 0:8], idt_sb[:], xt2[0][:, 0:8],
                            start=True, stop=True,
                        )
                    for kp in range(N_DBLK // 2):
                        if "proj_mm" in skip:
                            break
                        nc.tensor.matmul(
                            ps[:],
                            conv[kp][:, :, t * P:(t + 1) * P],
                            pwt_sb[:, 2 * kp:2 * kp + 2, :],
                            start=(kp == 0),
                            stop=False,
                            perf_mode=mybir.MatmulPerfMode.DoubleRow,
                        )
                    for k in range(N_DBLK):
                        if "proj_mm" in skip:
                            break
                        nc.tensor.matmul(
                            ps[:],
                            auto[k][:, t * P:(t + 1) * P],
                            pwb_sb[:, k, :],
                            start=False,
                            stop=False,
                        )
                    # residual: += x[tile].T via identity matmul (bf16).
                    # stop only on the last — the sim's psum group tracker
                    # clears started-state at zero-region granularity.
                    for q in range(N_DBLK):
                        if "proj_mm" in skip:
                            break
                        nc.tensor.matmul(
                            ps[:, q * P:(q + 1) * P],
                            xt2[q][:, SM + t * P:SM + (t + 1) * P],
                            idt_sb[:],
                            start=False,
                            stop=(q == N_DBLK - 1),
                        )
                    if need_pb:
                        nc.vector.tensor_tensor(ps[:], ps[:], pb_sb[:], Add)
                    ht = hpool.tile([P, D], BF16, tag="h")
                    nc.vector.tensor_scalar(
                        ht[:], ps[:], 1.0, 0.0, Mult, Add,
                        accum_out=hs[:, t:t + 1],
                    )
                    hts.append(ht)
                    jq = jpool.tile([P, D], BF16, tag="jact")
                    nc.scalar.activation(
                        jq[:], ps[:], Square, accum_out=hs2[:, t:t + 1]
                    )

                mu = tpool.tile([P, N_LT], F32, tag="mu")
                nc.vector.tensor_scalar(mu[:], hs[:], 1.0 / D, None, Mult)
                var = tpool.tile([P, N_LT], F32, tag="var")
                nc.vector.tensor_scalar(var[:], hs2[:], 1.0 / D, None, Mult)
                musq = tpool.tile([P, N_LT], F32, tag="musq")
                nc.vector.tensor_tensor(musq[:], mu[:], mu[:], Mult)
                nc.vector.tensor_tensor(var[:], var[:], musq[:], Sub)
                sd = tpool.tile([P, N_LT], F32, tag="sd")
                nc.scalar.activation(sd[:], var[:], Sqrt, bias=eps_sb[:])
                rstd = tpool.tile([P, N_LT], F32, tag="rstd")
                nc.vector.reciprocal(rstd[:], sd[:])
                oqs = (nc.sync, nc.scalar, nc.gpsimd)
                for g in range(N_LT // 4):
                    ot = opool.tile([P, 4, D], BF16, tag="out")
                    for u in range(4):
                        t = g * 4 + u
                        nc.gpsimd.tensor_scalar(
                            ot[:, u, :], hts[t][:],
                            mu[:, t:t + 1], rstd[:, t:t + 1],
                            Sub, Mult,
                        )
                        if need_gb:
                            nc.vector.tensor_tensor(
                                ot[:, u, :], ot[:, u, :], g_sb[:], Mult)
                            nc.vector.tensor_tensor(
                                ot[:, u, :], ot[:, u, :], bb_sb[:], Add)
                    if "out_dma" not in skip:
                        oqs[g % 3].dma_start(
                            out_d[b, g * 4 * P:(g + 1) * 4 * P, :].rearrange(
                                "(u p) d -> p u d", p=P
                            ),
                            ot[:],
                        )

            # software pipeline: A1(0) A2(0) A1(1) B(0) A2(1) A1(2) ...
            for _rep in range(repeat):
                sts = {0: emit_A(0)}
                sts[0]["rep"] = _rep
                emit_A2(0, sts[0])
                for b in range(1, B_LOC):
                    sts[b] = emit_A(b)
                    sts[b]["rep"] = _rep
                    emit_B(b - 1, sts.pop(b - 1))
                    emit_A2(b, sts[b])
                emit_B(B_LOC - 1, sts.pop(B_LOC - 1))

    nc.compile()
    return nc


_CACHE: dict = {}
tap_offsets: list = []


def _prepare(x, conv_w0, conv_w1, conv_w2, proj_w, proj_b, ln_g, ln_b):
    global tap_offsets
    x = np.asarray(x, dtype=np.float32)
    conv_w0 = np.asarray(conv_w0, np.float32)
    conv_w1 = np.asarray(conv_w1, np.float32)
    conv_w2 = np.asarray(conv_w2, np.float32)
    proj_w = np.asarray(proj_w, np.float32)
    proj_b = np.asarray(proj_b, np.float32)
    ln_g = np.asarray(ln_g, np.float32)
    ln_b = np.asarray(ln_b, np.float32)

    offs, tab = _tap_table(conv_w0, conv_w1, conv_w2)
    tap_offsets = list(offs)
    n_taps = len(tap_offsets)

    need_pb = bool(np.any(proj_b != 0.0))
    need_gb = bool(np.any(ln_g != 1.0) or np.any(ln_b != 0.0))
    key = (n_taps, need_pb, need_gb)

    lags = _cand_lags()
    tstart = (SM + L - lags.astype(np.int64)).astype(np.int32).reshape(1, MAXC)
    invcnt = (1.0 / ((L - lags.astype(np.float64)) * D * D)).astype(
        np.float32).reshape(MAXC, 1)

    xT = np.ascontiguousarray(x.transpose(0, 2, 1))
    xbf = xT.astype(ml_dtypes.bfloat16)
    xf8 = xT.astype(ml_dtypes.float8_e4m3)
    idt = np.eye(P, dtype=ml_dtypes.bfloat16)
    pwtT = np.ascontiguousarray(proj_w.T)
    pwt = pwtT[:D].astype(ml_dtypes.float8_e4m3)
    pwb = pwtT[D:].astype(ml_dtypes.bfloat16)
    # conv diag lhsT tiles: [n_taps * N_DBLK * P, P], row (j*N_DBLK+q)*P+p
    # holds diag(tap j weights for channel block q)
    cdg = np.zeros((n_taps * N_DBLK, P, P), np.float32)
    for j in range(n_taps):
        for q in range(N_DBLK):
            np.fill_diagonal(cdg[j * N_DBLK + q], tab[q * P:(q + 1) * P, j])
    cdg = cdg.reshape(n_taps * N_DBLK * P, P).astype(ml_dtypes.bfloat16)
    noncenter = [j for j in range(n_taps) if offs[j] != 0]
    noncenter.sort(key=lambda j: offs[j])
    n_pairs = len(noncenter) // 2
    cd8 = np.zeros((n_pairs * N_DBLK, P, 2, P), np.float32)
    for pi in range(n_pairs):
        j1, j2 = noncenter[2 * pi], noncenter[2 * pi + 1]
        for q in range(N_DBLK):
            np.fill_diagonal(cd8[pi * N_DBLK + q, :, 0, :],
                             tab[q * P:(q + 1) * P, j1])
            np.fill_diagonal(cd8[pi * N_DBLK + q, :, 1, :],
                             tab[q * P:(q + 1) * P, j2])
    cd8 = cd8.reshape(n_pairs * N_DBLK * P, 2 * P).astype(
        ml_dtypes.float8_e4m3)

    in_maps = []
    for c in range(N_CORES):
        m = {
            "xbf": np.ascontiguousarray(xbf[c * B_LOC:(c + 1) * B_LOC]),
            "xf8": np.ascontiguousarray(xf8[c * B_LOC:(c + 1) * B_LOC]),
            "cd8": cd8,
            "pwt": pwt,
            "pwb": pwb,
            "cdg": cdg,
            "tstart": tstart,
            "invcnt": invcnt,
            "idt": idt,
        }
        if need_pb:
            m["pb"] = proj_b.reshape(1, D)
        if need_gb:
            m["lng"] = ln_g.reshape(1, D)
            m["lnb"] = ln_b.reshape(1, D)
        in_maps.append(m)
    return key, in_maps


def kernel(x, conv_w0, conv_w1, conv_w2, proj_w, proj_b, ln_g, ln_b):
    key, in_maps = _prepare(
        x, conv_w0, conv_w1, conv_w2, proj_w, proj_b, ln_g, ln_b
    )
    if key not in _CACHE:
        _CACHE[key] = _build(*key)
    nc = _CACHE[key]
    res = run_bass_kernel_spmd(nc, in_maps, list(range(N_CORES)))
    out = np.concatenate([res.results[c]["out"] for c in range(N_CORES)], axis=0)
    return out.astype(np.float32)
